# revision 55
# baseline (speedup 1.0000x reference)
"""Full-width attention (B=4, S=2048, D=1024, no head split) on 8 TRN2 cores.

Sharding: data-parallel over (batch, query-half) -> 8 shards. Core c handles
batch b = c//2, query rows [h*1024, (h+1)*1024) with h = c%2.

Zero-redundancy decomposition (12.88 GFLOP/core, the 103 GFLOP/8 floor):
the weight fold Wq^T Wk is applied to the QUERY side, not the key side:
  qm   = x_own A          (A = Wq^T Wk)        2.15 GFLOP   [own 1024 q only]
  S^T  = x_full^T . qm^T  (contract raw e)     4.29 GFLOP   [keys need NO proj]
  E    = exp(S/8 + t3),   t3 = x.(Wk^T bq)     (host-computed, ACT bias)
  PX^T = x^T E            (contract k)         4.29 GFLOP
  out  = (PX/rowsum) Wv^T + bv                 2.15 GFLOP
Per-core inputs are key-permuted (own query half first) so the same SPMD
program can slice "own queries" at columns 0..1023; attention is invariant
to a consistent key permutation of (xT, xnb, t3).

Precision: scores path (A, xT, qm) in fp16 — fp16 matmuls run at full PE
rate (1 col/cycle, like bf16) but its 10 mantissa bits keep the softmax
input error ~0.3% l2, where bf16 would cost ~1.3% (1024-length
contractions; exp amplifies). fp16 range is fine there (all O(1) values);
the V path (xn, E, Px, Wv, out) needs bf16 range (E spans e^+-25) and
only averages errors, so it runs bf16. Everything 16-bit halves DMA/SBUF
and the whole value tensor stays resident: the PX phase does zero input
DMA. No warm-up junk: the first qm matmuls themselves ramp the HAM
clock-gate (cold productive work beats junk at half rate).

DMA structure (measured, not obvious): per-chunk 128-512 KiB transfers
with the first-needed chunks interleaved across the sync/gpsimd queues
beat few-big-transfer variants — Tile batches same-queue DMAs and gates
consumers on the whole batch, so big transfers delay the first matmul by
~8 us. A is laid out per-partition-contiguous (4 KiB bursts; the naive
layout's 512 B bursts are 4x slower and gated the first 42 us). Output
DMAs stay on the sync queue (HWDGE): SWDGE HBM-write completion receipts
add ~8 us to the final drain. out_pool is deep (12) because each
output-tile reuse waits on a ~2 us DMA completion receipt.

Measured: ~189 us on an idle chip (254 us baseline), l2 rel err 3.6e-3.
Chip-level power throttling (P0, ~2.0 GHz PE) adds ~18% in bad windows.
Known-fixed remainder: first matmul gated at ~12.2 us (consumer DMA gate
trails its dependency by ~5 same-queue slots at ~600 ns issue each, on
top of the ~7.2 us SPMD preamble); ~9.5 us tail (HBM-write receipt +
multi-core teardown barrier); ~2 us HAM cold-clock ramp.
"""

import math
from contextlib import ExitStack

import numpy as np

P = 128
B, S, D = 4, 2048, 1024
SQ = 1024  # query rows per core
KO = D // P  # 8 chunks of the d/e contraction dims
KC = S // P  # 16 key chunks
N_CORES = 8


def build_bass():
    from concourse import bacc
    import concourse.mybir as mybir
    from concourse.tile import TileContext

    f32 = mybir.dt.float32
    f32r = mybir.dt.float32r
    bf16 = mybir.dt.bfloat16
    f16 = mybir.dt.float16
    AF = mybir.ActivationFunctionType

    nc = bacc.Bacc(
        "TRN2",
        target_bir_lowering=False,
        debug=False,
        enable_asserts=False,
        num_devices=N_CORES,
    )

    xT = nc.dram_tensor("xT", [D, S], f16, kind="ExternalInput")
    a = nc.dram_tensor("a", [P, KO * D], f16, kind="ExternalInput")
    xnb = nc.dram_tensor("xnb", [S, D], bf16, kind="ExternalInput")
    wvT = nc.dram_tensor("wvT", [D, D], bf16, kind="ExternalInput")
    t3 = nc.dram_tensor("t3", [P, KC], f32, kind="ExternalInput")
    bvb = nc.dram_tensor("bvb", [P, D], f32, kind="ExternalInput")
    out = nc.dram_tensor("out", [SQ, D], bf16, kind="ExternalOutput")

    xT_r = xT[:, :].rearrange("(ko p) s -> p ko s", p=P)
    xnb_r = xnb[:, :].rearrange("(ko p) d -> p ko d", p=P)
    wvT_r = wvT[:, :].rearrange("(ko p) e -> p ko e", p=P)

    inv_sqrt_dk = 1.0 / math.sqrt(D // 16)  # d_key = 64

    with TileContext(nc) as tc, ExitStack() as ctx:
        xt_pool = ctx.enter_context(tc.tile_pool(name="xtp", bufs=1))
        qm_pool = ctx.enter_context(tc.tile_pool(name="qmp", bufs=1))
        msc_pool = ctx.enter_context(tc.tile_pool(name="msc", bufs=1))
        psA_p = ctx.enter_context(tc.tile_pool(name="psA", bufs=3, space="PSUM"))
        psB_p = ctx.enter_context(tc.tile_pool(name="psB", bufs=2, space="PSUM"))
        psC_p = ctx.enter_context(tc.tile_pool(name="psC", bufs=2, space="PSUM"))
        psR_p = ctx.enter_context(tc.tile_pool(name="psR", bufs=1, space="PSUM"))
        dram_p = ctx.enter_context(tc.tile_pool(name="drp", bufs=1, space="DRAM"))

        # raw x^T, resident; one tile per DMA so consumers gate precisely
        xt0a = xt_pool.tile([P, 4, 512], f16, tag="x0a", name="xt0a")
        xt0b = xt_pool.tile([P, 4, 512], f16, tag="x0b", name="xt0b")
        xt1 = xt_pool.tile([P, KO, 512], f16, tag="xt1", name="xt1")
        xt2 = xt_pool.tile([P, KO, 1024], f16, tag="xt2", name="xt2")

        def xt_qm(qc, ko):
            # qm moving operand for own-query chunk qc, feature chunk ko
            if qc == 1:
                return xt1[:, ko, :]
            return xt0a[:, ko, :] if ko < 4 else xt0b[:, ko - 4, :]

        def xt_sc(kc, eo):
            # scores stationary operand for key chunk kc, feature chunk eo
            kcol = (kc % 4) * P
            if kc < 4:
                t = xt0a if eo < 4 else xt0b
                return t[:, eo % 4, kcol : kcol + P]
            if kc < 8:
                return xt1[:, eo, kcol : kcol + P]
            kcol = (kc - 8) * P
            return xt2[:, eo, kcol : kcol + P]

        qmT = qm_pool.tile([P, KO, SQ], f16)  # (x_own A)^T, resident

        # PE warm-up tile generated on-chip (iota + cast): no DMA dependency,
        # so the HAM activity window opens while the first operands stream in.
        iti = msc_pool.tile([P, 256], mybir.dt.int32, tag="iti", name="iti")
        nc.gpsimd.iota(iti[:], pattern=[[0, 256]], base=1, channel_multiplier=0)
        warm = msc_pool.tile([P, 256], f32r, tag="warm", name="warm")
        nc.vector.tensor_copy(warm[:], iti[:])
        t3_t = msc_pool.tile([P, KC], f32, tag="t3t", name="t3_t")

        # ---- Phase Q: qm^T[e, q] = A^T x_own^T (A resident, phase-scoped) ----
        with tc.tile_pool(name="ap", bufs=1) as a_pool:
            # few BIG transfers (queue issue costs ~600ns per dma_start, so
            # the head is count-bound) — BUT a consumer's DMA-sem threshold
            # covers every same-queue dma_start issued earlier in program
            # order, so each transfer is issued right before its consumers
            a_q0 = a_pool.tile([P, 2 * D], f16, tag="aq0", name="a_q0")
            a_q1 = a_pool.tile([P, 6 * D], f16, tag="aq1", name="a_q1")
            # first chunks extra-small: the consumer gate trails its true
            # dependency by ~5 same-queue DMA slots, so early slot DURATION
            # sets the first-matmul time
            nc.sync.dma_start(a_q0[:, 0 : D // 2], a[:, 0 : D // 2])
            for ko in range(4):
                nc.gpsimd.dma_start(xt0a[:, ko, :], xT_r[:, ko, 0:512])
            nc.scalar.dma_start(xt0b[:, :, :], xT_r[:, 4:8, 0:512])
            nc.sync.dma_start(a_q0[:, D // 2 : D], a[:, D // 2 : D])
            nc.sync.dma_start(a_q0[:, D : 2 * D], a[:, D : 2 * D])
            for eo in range(2, KO):
                # alternate queues: shortens each block's trailing DMA gate
                q = nc.scalar if eo % 2 == 0 else nc.sync
                q.dma_start(
                    a_q1[:, (eo - 2) * D : (eo - 1) * D], a[:, eo * D : (eo + 1) * D]
                )
            for ko in range(0, KO, 4):
                nc.gpsimd.dma_start(
                    xt1[:, ko : ko + 4, :], xT_r[:, ko : ko + 4, 512:1024]
                )
            for ko in range(0, KO, 4):
                nc.gpsimd.dma_start(
                    xt2[:, ko : ko + 4, :], xT_r[:, ko : ko + 4, 1024:2048]
                )

            def a_sl(eo, ko):
                if eo < 2:
                    return a_q0[:, (eo * KO + ko) * P : (eo * KO + ko + 1) * P]
                i = (eo - 2) * KO + ko
                return a_q1[:, i * P : (i + 1) * P]

            for qc in range(2):
                for eo in range(KO):
                    pa = psA_p.tile([P, 512], f32, tag="psA", name="paq")
                    for ko in range(KO):
                        nc.tensor.matmul(
                            pa[:], a_sl(eo, ko), xt_qm(qc, ko),
                            start=(ko == 0), stop=(ko == KO - 1),
                        )
                    nc.scalar.copy(qmT[:, eo, qc * 512 : (qc + 1) * 512], pa[:])
                if qc == 0:
                    nc.sync.dma_start(t3_t[:], t3[:, :])

        # ---------------- Phase C: attention ----------------
        with (
            tc.tile_pool(name="ep", bufs=1) as e_pool,
            tc.tile_pool(name="vsp", bufs=1) as vs_pool,
            tc.tile_pool(name="osp", bufs=12) as out_pool,
        ):
            xnb_t = vs_pool.tile([P, KC, D], bf16, tag="xnb", name="xnb_t")
            nc.gpsimd.dma_start(xnb_t[:, 0:8, :], xnb_r[:, 0:8, :])
            nc.gpsimd.dma_start(xnb_t[:, 8:16, :], xnb_r[:, 8:16, :])
            wv_sb = [
                vs_pool.tile([P, KO, 512], bf16, tag=f"wv{h}", name=f"wv_sb{h}")
                for h in range(2)
            ]
            for h in range(2):
                nc.gpsimd.dma_start(
                    wv_sb[h][:, :, :], wvT_r[:, :, h * 512 : (h + 1) * 512]
                )
            bvb_t = msc_pool.tile([P, D], f32, tag="bvb", name="bvb_t")
            nc.gpsimd.dma_start(bvb_t[:], bvb[:, :])
            pxt_sb = vs_pool.tile([P, KO, 512], bf16, tag="pxt", name="pxt_sb")

            for qc in range(2):
                E = e_pool.tile([P, KC, 512], bf16, tag="E", name="E")
                racc = msc_pool.tile([P, 512], f32r, tag="racc", name="racc")
                for kc in range(KC):
                    pa = psA_p.tile([P, 512], f32, tag="psA", name="pas")
                    for eo in range(KO):
                        nc.tensor.matmul(
                            pa[:],
                            xt_sc(kc, eo),
                            qmT[:, eo, qc * 512 : (qc + 1) * 512],
                            start=(eo == 0),
                            stop=(eo == KO - 1),
                        )
                    nc.scalar.activation(
                        E[:, kc, :], pa[:], AF.Exp, scale=inv_sqrt_dk,
                        bias=t3_t[:, kc : kc + 1],
                    )
                    if kc == 0:
                        nc.vector.tensor_copy(racc[:], E[:, 0, :])
                    else:
                        nc.vector.tensor_add(racc[:], racc[:], E[:, kc, :])
                # partition-reduce rowsum with one ones-matmul, then
                # [1,512] -> per-partition recips [128,4] via DRAM bounce
                pr = psR_p.tile([1, 512], f32, tag="psR", name="pr")
                nc.tensor.matmul(pr[:], warm[:, 0:1], racc[:])
                rsum_row = msc_pool.tile([1, 512], f32, tag="rsr", name="rsum_row")
                nc.scalar.copy(rsum_row[:], pr[:])
                rs_dram = dram_p.tile([1, 512], f32, tag="rsd", name="rs_dram")
                nc.sync.dma_start(rs_dram[:, :], rsum_row[:, :])
                rsum_t = msc_pool.tile([P, 4], f32, tag="rst", name="rsum_t")
                nc.sync.dma_start(
                    rsum_t[:, :], rs_dram[0, :].rearrange("(qs p) -> p qs", p=P)
                )
                recip = msc_pool.tile([P, 4], f32, tag="recip", name="recip")
                nc.vector.reciprocal(recip[:], rsum_t[:])

                # PX^T[d, q] = sum_k x[k, d] E[k, q]: fully SBUF-fed (bf16).
                # bank order: outMM consumes psB/psC first, so evac them first
                pxt_ps = [
                    psB_p.tile([P, 512], f32, tag="psB", name="px0"),
                    psC_p.tile([P, 512], f32, tag="psC", name="px1"),
                    psB_p.tile([P, 512], f32, tag="psB", name="px2"),
                    psC_p.tile([P, 512], f32, tag="psC", name="px3"),
                    psA_p.tile([P, 512], f32, tag="psA", name="px4"),
                    psA_p.tile([P, 512], f32, tag="psA", name="px5"),
                    psA_p.tile([P, 512], f32, tag="psA", name="px6"),
                    psR_p.tile([P, 512], f32, tag="psR", name="px7"),
                ]
                # dc-outer (xnb is resident, no per-ko DMA reuse to respect):
                # each PSUM tile completes early and its evac overlaps the
                # next tile's matmuls instead of trailing the whole phase
                for dc in range(KO):
                    for ko in range(KC):
                        nc.tensor.matmul(
                            pxt_ps[dc][:],
                            xnb_t[:, ko, dc * P : (dc + 1) * P],
                            E[:, ko, :],
                            start=(ko == 0),
                            stop=(ko == KC - 1),
                        )
                    nc.vector.tensor_copy(pxt_sb[:, dc, :], pxt_ps[dc][:])
                # out[q, e] = PX^T.T @ Wv^T, scaled by 1/rowsum, + bv
                for qs in range(4):
                    pb = psB_p.tile([P, 512], f32, tag="psB", name="avb")
                    pc = psC_p.tile([P, 512], f32, tag="psC", name="avc")
                    # pb fully first: its ACT/add/DMA chain then overlaps
                    # pc's matmuls instead of trailing the very last MM
                    for ko in range(KO):
                        nc.tensor.matmul(
                            pb[:], pxt_sb[:, ko, qs * P : (qs + 1) * P],
                            wv_sb[0][:, ko, :],
                            start=(ko == 0), stop=(ko == KO - 1),
                        )
                    for ko in range(KO):
                        nc.tensor.matmul(
                            pc[:], pxt_sb[:, ko, qs * P : (qs + 1) * P],
                            wv_sb[1][:, ko, :],
                            start=(ko == 0), stop=(ko == KO - 1),
                        )
                    row0 = qc * 512 + qs * P
                    for quarter in range(4):
                        ps = pb if quarter < 2 else pc
                        c0 = quarter * 256
                        o = out_pool.tile([P, 256], bf16, tag="ost", name="ost")
                        nc.scalar.activation(
                            o[:], ps[:, (quarter % 2) * 256 : (quarter % 2) * 256 + 256],
                            AF.Identity, scale=recip[:, qs : qs + 1],
                        )
                        nc.vector.tensor_add(o[:], o[:], bvb_t[:, c0 : c0 + 256])
                        nc.sync.dma_start(out[row0 : row0 + P, c0 : c0 + 256], o[:])

    nc.finalize()
    return nc


def make_in_maps(x, Wq, bq, Wk, bk, Wv, bv):
    """Build the 8 per-core input maps from full inputs."""
    import ml_dtypes

    bf16 = ml_dtypes.bfloat16
    x = np.asarray(x, dtype=np.float32)
    # weight-only folding: scores = x (Wq^T Wk) x^T + per-key bias x.(Wk^T bq)
    # (+ per-query consts, softmax-invariant, dropped)
    A = (np.asarray(Wq, np.float64).T @ np.asarray(Wk, np.float64)).astype(np.float32)
    # per-partition-contiguous layout: a_pre[p, eo*1024 + ko*128 + e]
    #   = A[ko*128 + p, eo*128 + e] -> each partition reads 4 KiB bursts
    a_pre = np.ascontiguousarray(
        A.reshape(KO, P, KO, P).transpose(1, 2, 0, 3).reshape(P, KO * D)
    ).astype(np.float16)
    wvT = np.ascontiguousarray(np.asarray(Wv, np.float32).T.astype(bf16))
    w3 = (np.asarray(Wk, np.float64).T @ np.asarray(bq, np.float64)).astype(np.float32)
    inv_sqrt_dk = 1.0 / math.sqrt(D // 16)
    bvb = np.ascontiguousarray(np.broadcast_to(np.asarray(bv, np.float32), (P, D)))

    in_maps = []
    for c in range(N_CORES):
        b, h = c // 2, c % 2
        # key-permute so the core's own query half sits at columns/rows 0..1023
        perm = (
            np.arange(S)
            if h == 0
            else np.concatenate([np.arange(SQ, S), np.arange(0, SQ)])
        )
        xp = x[b][perm]  # [S, D], rows permuted
        t3v = (xp @ w3) * inv_sqrt_dk  # [S]
        in_maps.append(
            {
                "xT": np.ascontiguousarray(xp.T.astype(np.float16)),
                "a": a_pre,
                "xnb": np.ascontiguousarray(xp.astype(bf16)),
                "wvT": wvT,
                "t3": np.ascontiguousarray(t3v.reshape(KC, P).T),
                "bvb": bvb,
            }
        )
    return in_maps


_NC_CACHE = None


def get_nc():
    global _NC_CACHE
    if _NC_CACHE is None:
        _NC_CACHE = build_bass()
    return _NC_CACHE


def kernel(x, Wq, bq, Wk, bk, Wv, bv, **run_kwargs):
    from concourse.bass_utils import run_bass_kernel_spmd

    nc = get_nc()
    in_maps = make_in_maps(x, Wq, bq, Wk, bk, Wv, bv)
    res = run_bass_kernel_spmd(
        nc, in_maps, core_ids=list(range(N_CORES)), **run_kwargs
    )
    out = np.empty((B, S, D), dtype=np.float32)
    for c in range(N_CORES):
        b, h = c // 2, c % 2
        out[b, h * SQ : (h + 1) * SQ, :] = np.asarray(
            res.results[c]["out"], dtype=np.float32
        )
    if run_kwargs.get("trace"):
        kernel.last_results = res
    return out


# revision 56
# speedup vs baseline: 1.0103x; 1.0103x over previous
"""Full-width attention (B=4, S=2048, D=1024, no head split) on 8 TRN2 cores.

Sharding: data-parallel over (batch, query-half) -> 8 shards. Core c handles
batch b = c//2, query rows [h*1024, (h+1)*1024) with h = c%2.

Zero-redundancy decomposition (12.88 GFLOP/core, the 103 GFLOP/8 floor):
the weight fold Wq^T Wk is applied to the QUERY side, not the key side:
  qm   = x_own A          (A = Wq^T Wk)        2.15 GFLOP   [own 1024 q only]
  S^T  = x_full^T . qm^T  (contract raw e)     4.29 GFLOP   [keys need NO proj]
  E    = exp(S/8 + t3),   t3 = x.(Wk^T bq)     (host-computed, ACT bias)
  PX^T = x^T E            (contract k)         4.29 GFLOP
  out  = (PX/rowsum) Wv^T + bv                 2.15 GFLOP
Per-core inputs are key-permuted (own query half first) so the same SPMD
program can slice "own queries" at columns 0..1023; attention is invariant
to a consistent key permutation of (xT, xnb, t3).

Precision: scores path (A, xT, qm) in fp16 — fp16 matmuls run at full PE
rate (1 col/cycle, like bf16) but its 10 mantissa bits keep the softmax
input error ~0.3% l2, where bf16 would cost ~1.3% (1024-length
contractions; exp amplifies). fp16 range is fine there (all O(1) values);
the V path (xn, E, Px, Wv, out) needs bf16 range (E spans e^+-25) and
only averages errors, so it runs bf16. Everything 16-bit halves DMA/SBUF
and the whole value tensor stays resident: the PX phase does zero input
DMA. No warm-up junk: the first qm matmuls themselves ramp the HAM
clock-gate (cold productive work beats junk at half rate).

DMA structure (measured, not obvious): per-chunk 128-512 KiB transfers
with the first-needed chunks interleaved across the sync/gpsimd queues
beat few-big-transfer variants — Tile batches same-queue DMAs and gates
consumers on the whole batch, so big transfers delay the first matmul by
~8 us. A is laid out per-partition-contiguous (4 KiB bursts; the naive
layout's 512 B bursts are 4x slower and gated the first 42 us). Output
DMAs stay on the sync queue (HWDGE): SWDGE HBM-write completion receipts
add ~8 us to the final drain. out_pool is deep (12) because each
output-tile reuse waits on a ~2 us DMA completion receipt.

Measured: ~189 us on an idle chip (254 us baseline), l2 rel err 3.6e-3.
Chip-level power throttling (P0, ~2.0 GHz PE) adds ~18% in bad windows.
Known-fixed remainder: first matmul gated at ~12.2 us (consumer DMA gate
trails its dependency by ~5 same-queue slots at ~600 ns issue each, on
top of the ~7.2 us SPMD preamble); ~9.5 us tail (HBM-write receipt +
multi-core teardown barrier); ~2 us HAM cold-clock ramp.
"""

import math
from contextlib import ExitStack

import numpy as np

P = 128
B, S, D = 4, 2048, 1024
SQ = 1024  # query rows per core
KO = D // P  # 8 chunks of the d/e contraction dims
KC = S // P  # 16 key chunks
N_CORES = 8


def build_bass():
    from concourse import bacc
    import concourse.mybir as mybir
    from concourse.tile import TileContext

    f32 = mybir.dt.float32
    f32r = mybir.dt.float32r
    bf16 = mybir.dt.bfloat16
    f16 = mybir.dt.float16
    AF = mybir.ActivationFunctionType

    nc = bacc.Bacc(
        "TRN2",
        target_bir_lowering=False,
        debug=False,
        enable_asserts=False,
        num_devices=N_CORES,
    )

    xT = nc.dram_tensor("xT", [D, S], f16, kind="ExternalInput")
    a = nc.dram_tensor("a", [P, KO * D], f16, kind="ExternalInput")
    xnb = nc.dram_tensor("xnb", [S, D], bf16, kind="ExternalInput")
    wvT = nc.dram_tensor("wvT", [D, D], bf16, kind="ExternalInput")
    t3 = nc.dram_tensor("t3", [P, KC], f32, kind="ExternalInput")
    bvb = nc.dram_tensor("bvb", [P, D], f32, kind="ExternalInput")
    out = nc.dram_tensor("out", [SQ, D], bf16, kind="ExternalOutput")

    xT_r = xT[:, :].rearrange("(ko p) s -> p ko s", p=P)
    xnb_r = xnb[:, :].rearrange("(ko p) d -> p ko d", p=P)
    wvT_r = wvT[:, :].rearrange("(ko p) e -> p ko e", p=P)

    inv_sqrt_dk = 1.0 / math.sqrt(D // 16)  # d_key = 64

    with TileContext(nc) as tc, ExitStack() as ctx:
        xt_pool = ctx.enter_context(tc.tile_pool(name="xtp", bufs=1))
        qm_pool = ctx.enter_context(tc.tile_pool(name="qmp", bufs=1))
        msc_pool = ctx.enter_context(tc.tile_pool(name="msc", bufs=1))
        psA_p = ctx.enter_context(tc.tile_pool(name="psA", bufs=3, space="PSUM"))
        psB_p = ctx.enter_context(tc.tile_pool(name="psB", bufs=2, space="PSUM"))
        psC_p = ctx.enter_context(tc.tile_pool(name="psC", bufs=2, space="PSUM"))
        psR_p = ctx.enter_context(tc.tile_pool(name="psR", bufs=1, space="PSUM"))
        dram_p = ctx.enter_context(tc.tile_pool(name="drp", bufs=1, space="DRAM"))

        # raw x^T, resident; one tile per DMA so consumers gate precisely
        xt0a = xt_pool.tile([P, 4, 512], f16, tag="x0a", name="xt0a")
        xt0b = xt_pool.tile([P, 4, 512], f16, tag="x0b", name="xt0b")
        xt1 = xt_pool.tile([P, KO, 512], f16, tag="xt1", name="xt1")
        xt2 = xt_pool.tile([P, KO, 1024], f16, tag="xt2", name="xt2")

        def xt_qm(qc, ko):
            # qm moving operand for own-query chunk qc, feature chunk ko
            if qc == 1:
                return xt1[:, ko, :]
            return xt0a[:, ko, :] if ko < 4 else xt0b[:, ko - 4, :]

        def xt_sc(kc, eo):
            # scores stationary operand for key chunk kc, feature chunk eo
            kcol = (kc % 4) * P
            if kc < 4:
                t = xt0a if eo < 4 else xt0b
                return t[:, eo % 4, kcol : kcol + P]
            if kc < 8:
                return xt1[:, eo, kcol : kcol + P]
            kcol = (kc - 8) * P
            return xt2[:, eo, kcol : kcol + P]

        qmT = qm_pool.tile([P, KO, SQ], f16)  # (x_own A)^T, resident

        # PE warm-up tile generated on-chip (iota + cast): no DMA dependency,
        # so the HAM activity window opens while the first operands stream in.
        iti = msc_pool.tile([P, 256], mybir.dt.int32, tag="iti", name="iti")
        nc.gpsimd.iota(iti[:], pattern=[[0, 256]], base=1, channel_multiplier=0)
        warm = msc_pool.tile([P, 256], f32r, tag="warm", name="warm")
        nc.vector.tensor_copy(warm[:], iti[:])
        t3_t = msc_pool.tile([P, KC], f32, tag="t3t", name="t3_t")

        # ---- Phase Q: qm^T[e, q] = A^T x_own^T (A resident, phase-scoped) ----
        with tc.tile_pool(name="ap", bufs=1) as a_pool:
            # few BIG transfers (queue issue costs ~600ns per dma_start, so
            # the head is count-bound) — BUT a consumer's DMA-sem threshold
            # covers every same-queue dma_start issued earlier in program
            # order, so each transfer is issued right before its consumers
            a_q0 = a_pool.tile([P, 2 * D], f16, tag="aq0", name="a_q0")
            a_q1 = a_pool.tile([P, 6 * D], f16, tag="aq1", name="a_q1")
            # first chunks extra-small: the consumer gate trails its true
            # dependency by ~5 same-queue DMA slots, so early slot DURATION
            # sets the first-matmul time
            nc.sync.dma_start(a_q0[:, 0 : D // 2], a[:, 0 : D // 2])
            for ko in range(4):
                nc.gpsimd.dma_start(xt0a[:, ko, :], xT_r[:, ko, 0:512])
            nc.scalar.dma_start(xt0b[:, :, :], xT_r[:, 4:8, 0:512])
            nc.sync.dma_start(a_q0[:, D // 2 : D], a[:, D // 2 : D])
            nc.sync.dma_start(a_q0[:, D : 2 * D], a[:, D : 2 * D])
            for eo in range(2, KO):
                nc.sync.dma_start(
                    a_q1[:, (eo - 2) * D : (eo - 1) * D], a[:, eo * D : (eo + 1) * D]
                )
            for ko in range(0, KO, 4):
                nc.gpsimd.dma_start(
                    xt1[:, ko : ko + 4, :], xT_r[:, ko : ko + 4, 512:1024]
                )
            for ko in range(0, KO, 4):
                nc.gpsimd.dma_start(
                    xt2[:, ko : ko + 4, :], xT_r[:, ko : ko + 4, 1024:2048]
                )

            def a_sl(eo, ko):
                if eo < 2:
                    return a_q0[:, (eo * KO + ko) * P : (eo * KO + ko + 1) * P]
                i = (eo - 2) * KO + ko
                return a_q1[:, i * P : (i + 1) * P]

            for qc in range(2):
                for eo in range(KO):
                    pa = psA_p.tile([P, 512], f32, tag="psA", name="paq")
                    for ko in range(KO):
                        nc.tensor.matmul(
                            pa[:], a_sl(eo, ko), xt_qm(qc, ko),
                            start=(ko == 0), stop=(ko == KO - 1),
                        )
                    nc.scalar.copy(qmT[:, eo, qc * 512 : (qc + 1) * 512], pa[:])
                if qc == 0:
                    nc.sync.dma_start(t3_t[:], t3[:, :])

        # ---------------- Phase C: attention ----------------
        with (
            tc.tile_pool(name="ep", bufs=1) as e_pool,
            tc.tile_pool(name="vsp", bufs=1) as vs_pool,
            tc.tile_pool(name="osp", bufs=12) as out_pool,
        ):
            xnb_t = vs_pool.tile([P, KC, D], bf16, tag="xnb", name="xnb_t")
            nc.gpsimd.dma_start(xnb_t[:, 0:8, :], xnb_r[:, 0:8, :])
            nc.gpsimd.dma_start(xnb_t[:, 8:16, :], xnb_r[:, 8:16, :])
            wv_sb = [
                vs_pool.tile([P, KO, 512], bf16, tag=f"wv{h}", name=f"wv_sb{h}")
                for h in range(2)
            ]
            for h in range(2):
                nc.gpsimd.dma_start(
                    wv_sb[h][:, :, :], wvT_r[:, :, h * 512 : (h + 1) * 512]
                )
            bvb_t = msc_pool.tile([P, D], f32, tag="bvb", name="bvb_t")
            nc.gpsimd.dma_start(bvb_t[:], bvb[:, :])
            pxt_sb = vs_pool.tile([P, KO, 512], bf16, tag="pxt", name="pxt_sb")

            for qc in range(2):
                E = e_pool.tile([P, KC, 512], bf16, tag="E", name="E")
                racc = msc_pool.tile([P, 512], f32r, tag="racc", name="racc")
                for kc in range(KC):
                    pa = psA_p.tile([P, 512], f32, tag="psA", name="pas")
                    for eo in range(KO):
                        nc.tensor.matmul(
                            pa[:],
                            xt_sc(kc, eo),
                            qmT[:, eo, qc * 512 : (qc + 1) * 512],
                            start=(eo == 0),
                            stop=(eo == KO - 1),
                        )
                    nc.scalar.activation(
                        E[:, kc, :], pa[:], AF.Exp, scale=inv_sqrt_dk,
                        bias=t3_t[:, kc : kc + 1],
                    )
                    if kc == 0:
                        nc.vector.tensor_copy(racc[:], E[:, 0, :])
                    else:
                        nc.vector.tensor_add(racc[:], racc[:], E[:, kc, :])
                # partition-reduce rowsum with one ones-matmul, then
                # [1,512] -> per-partition recips [128,4] via DRAM bounce
                pr = psR_p.tile([1, 512], f32, tag="psR", name="pr")
                nc.tensor.matmul(pr[:], warm[:, 0:1], racc[:])
                rsum_row = msc_pool.tile([1, 512], f32, tag="rsr", name="rsum_row")
                nc.scalar.copy(rsum_row[:], pr[:])
                rs_dram = dram_p.tile([1, 512], f32, tag="rsd", name="rs_dram")
                nc.sync.dma_start(rs_dram[:, :], rsum_row[:, :])
                rsum_t = msc_pool.tile([P, 4], f32, tag="rst", name="rsum_t")
                nc.sync.dma_start(
                    rsum_t[:, :], rs_dram[0, :].rearrange("(qs p) -> p qs", p=P)
                )
                recip = msc_pool.tile([P, 4], f32, tag="recip", name="recip")
                nc.vector.reciprocal(recip[:], rsum_t[:])

                # PX^T[d, q] = sum_k x[k, d] E[k, q]: fully SBUF-fed (bf16).
                # bank order: outMM consumes psB/psC first, so evac them first
                pxt_ps = [
                    psB_p.tile([P, 512], f32, tag="psB", name="px0"),
                    psC_p.tile([P, 512], f32, tag="psC", name="px1"),
                    psB_p.tile([P, 512], f32, tag="psB", name="px2"),
                    psC_p.tile([P, 512], f32, tag="psC", name="px3"),
                    psA_p.tile([P, 512], f32, tag="psA", name="px4"),
                    psA_p.tile([P, 512], f32, tag="psA", name="px5"),
                    psA_p.tile([P, 512], f32, tag="psA", name="px6"),
                    psR_p.tile([P, 512], f32, tag="psR", name="px7"),
                ]
                # dc-outer (xnb is resident, no per-ko DMA reuse to respect):
                # each PSUM tile completes early and its evac overlaps the
                # next tile's matmuls instead of trailing the whole phase
                for dc in range(KO):
                    for ko in range(KC):
                        nc.tensor.matmul(
                            pxt_ps[dc][:],
                            xnb_t[:, ko, dc * P : (dc + 1) * P],
                            E[:, ko, :],
                            start=(ko == 0),
                            stop=(ko == KC - 1),
                        )
                    nc.vector.tensor_copy(pxt_sb[:, dc, :], pxt_ps[dc][:])
                # out[q, e] = PX^T.T @ Wv^T, scaled by 1/rowsum, + bv
                for qs in range(4):
                    pb = psB_p.tile([P, 512], f32, tag="psB", name="avb")
                    pc = psC_p.tile([P, 512], f32, tag="psC", name="avc")
                    # pb fully first: its ACT/add/DMA chain then overlaps
                    # pc's matmuls instead of trailing the very last MM
                    for ko in range(KO):
                        nc.tensor.matmul(
                            pb[:], pxt_sb[:, ko, qs * P : (qs + 1) * P],
                            wv_sb[0][:, ko, :],
                            start=(ko == 0), stop=(ko == KO - 1),
                        )
                    for ko in range(KO):
                        nc.tensor.matmul(
                            pc[:], pxt_sb[:, ko, qs * P : (qs + 1) * P],
                            wv_sb[1][:, ko, :],
                            start=(ko == 0), stop=(ko == KO - 1),
                        )
                    row0 = qc * 512 + qs * P
                    for quarter in range(4):
                        ps = pb if quarter < 2 else pc
                        c0 = quarter * 256
                        o = out_pool.tile([P, 256], bf16, tag="ost", name="ost")
                        nc.scalar.activation(
                            o[:], ps[:, (quarter % 2) * 256 : (quarter % 2) * 256 + 256],
                            AF.Identity, scale=recip[:, qs : qs + 1],
                        )
                        nc.vector.tensor_add(o[:], o[:], bvb_t[:, c0 : c0 + 256])
                        nc.sync.dma_start(out[row0 : row0 + P, c0 : c0 + 256], o[:])

    nc.finalize()
    return nc


def make_in_maps(x, Wq, bq, Wk, bk, Wv, bv):
    """Build the 8 per-core input maps from full inputs."""
    import ml_dtypes

    bf16 = ml_dtypes.bfloat16
    x = np.asarray(x, dtype=np.float32)
    # weight-only folding: scores = x (Wq^T Wk) x^T + per-key bias x.(Wk^T bq)
    # (+ per-query consts, softmax-invariant, dropped)
    A = (np.asarray(Wq, np.float64).T @ np.asarray(Wk, np.float64)).astype(np.float32)
    # per-partition-contiguous layout: a_pre[p, eo*1024 + ko*128 + e]
    #   = A[ko*128 + p, eo*128 + e] -> each partition reads 4 KiB bursts
    a_pre = np.ascontiguousarray(
        A.reshape(KO, P, KO, P).transpose(1, 2, 0, 3).reshape(P, KO * D)
    ).astype(np.float16)
    wvT = np.ascontiguousarray(np.asarray(Wv, np.float32).T.astype(bf16))
    w3 = (np.asarray(Wk, np.float64).T @ np.asarray(bq, np.float64)).astype(np.float32)
    inv_sqrt_dk = 1.0 / math.sqrt(D // 16)
    bvb = np.ascontiguousarray(np.broadcast_to(np.asarray(bv, np.float32), (P, D)))

    in_maps = []
    for c in range(N_CORES):
        b, h = c // 2, c % 2
        # key-permute so the core's own query half sits at columns/rows 0..1023
        perm = (
            np.arange(S)
            if h == 0
            else np.concatenate([np.arange(SQ, S), np.arange(0, SQ)])
        )
        xp = x[b][perm]  # [S, D], rows permuted
        t3v = (xp @ w3) * inv_sqrt_dk  # [S]
        in_maps.append(
            {
                "xT": np.ascontiguousarray(xp.T.astype(np.float16)),
                "a": a_pre,
                "xnb": np.ascontiguousarray(xp.astype(bf16)),
                "wvT": wvT,
                "t3": np.ascontiguousarray(t3v.reshape(KC, P).T),
                "bvb": bvb,
            }
        )
    return in_maps


_NC_CACHE = None


def get_nc():
    global _NC_CACHE
    if _NC_CACHE is None:
        _NC_CACHE = build_bass()
    return _NC_CACHE


def kernel(x, Wq, bq, Wk, bk, Wv, bv, **run_kwargs):
    from concourse.bass_utils import run_bass_kernel_spmd

    nc = get_nc()
    in_maps = make_in_maps(x, Wq, bq, Wk, bk, Wv, bv)
    res = run_bass_kernel_spmd(
        nc, in_maps, core_ids=list(range(N_CORES)), **run_kwargs
    )
    out = np.empty((B, S, D), dtype=np.float32)
    for c in range(N_CORES):
        b, h = c // 2, c % 2
        out[b, h * SQ : (h + 1) * SQ, :] = np.asarray(
            res.results[c]["out"], dtype=np.float32
        )
    if run_kwargs.get("trace"):
        kernel.last_results = res
    return out
